# revision 5
# baseline (speedup 1.0000x reference)
"""Trainium2 Bass kernel for the BettingLoss problem.

Strategy (pure data parallel, 8 NeuronCores):
  - Shard the batch dim of the three [B, 6] f32 tensors into 8 contiguous
    row blocks, one per core.
  - Per core, stream tiles of [128 partitions x R rows x 6 traps] through
    SBUF and compute, per row:
        a_t  = fl(fl(odds_t * 1.1) * probs_t)        (matches reference op order)
        wo_t = fl(win_t * odds_t)
        running first-max select chain over t=0..5 tracking (best_a, wo_at_best)
        bet  = best_a > 1.0     (exactly equivalent to reference's ep > 0 test)
    and accumulate three per-partition sums with free accum_out outputs:
        S_A  = sum bet * best_a
        S_WO = sum bet * wo_at_best
        NB   = sum bet
  - Host combines the per-core/per-partition partials in float64:
        sum_bet_ep   = 0.019 * (S_A - NB)
        loss         = -sum_bet_ep / B          (fallback if NB == 0)
        batch_profit = (1.1*0.02*0.95) * S_WO - (0.02*0.95) * NB
        num_bets     = NB
"""

import sys

if "/opt/trn_rl_repo" not in sys.path:
    sys.path.insert(0, "/opt/trn_rl_repo")

import numpy as np

B = 4_194_304
T = 6
N_CORES = 8
BC = B // N_CORES          # rows per core
P = 128                    # SBUF partitions
ROWS_PP = BC // P          # rows per partition per core (4096)
R = 1024                   # rows per partition per tile
NT = ROWS_PP // R          # tiles per core
F = T * R                  # free-dim elements per input tile

ALPHA = 1.1
COMMISSION = 0.05
BET_PCT = 0.02
PAYOUT_SCALE = BET_PCT * (1.0 - COMMISSION)          # 0.019
WIN_COEF = ALPHA * BET_PCT * (1.0 - COMMISSION)      # 0.0209

_PROGRAM = None


def _build_program():
    from concourse import bacc, mybir
    from concourse.tile import TileContext

    DT = mybir.dt.float32
    Alu = mybir.AluOpType

    nc = bacc.Bacc("TRN2", target_bir_lowering=False, debug=False,
                   num_devices=N_CORES)
    probs_d = nc.dram_tensor("probs", [BC, T], DT, kind="ExternalInput").ap()
    odds_d = nc.dram_tensor("odds", [BC, T], DT, kind="ExternalInput").ap()
    win_d = nc.dram_tensor("win", [BC, T], DT, kind="ExternalInput").ap()
    acc_d = nc.dram_tensor("acc", [P, 3 * NT], DT, kind="ExternalOutput").ap()

    # Partition-major contiguous row blocks: partition p holds rows
    # [p*ROWS_PP, (p+1)*ROWS_PP) of this core's shard.
    pv = probs_d.rearrange("(p n) t -> p (n t)", p=P)
    ov = odds_d.rearrange("(p n) t -> p (n t)", p=P)
    wv = win_d.rearrange("(p n) t -> p (n t)", p=P)

    with TileContext(nc) as tc:
        with tc.tile_pool(name="io", bufs=2) as io_pool, \
             tc.tile_pool(name="chain", bufs=1) as ch_pool, \
             tc.tile_pool(name="wobp", bufs=2) as wob_pool, \
             tc.tile_pool(name="accp", bufs=1) as acc_pool:
            acc = acc_pool.tile([P, 3 * NT], DT)
            for k in range(NT):
                pt = io_pool.tile([P, F], DT, tag="pt")
                ot = io_pool.tile([P, F], DT, tag="ot")
                wt = io_pool.tile([P, F], DT, tag="wt")
                nc.sync.dma_start(out=pt[:], in_=pv[:, k * F:(k + 1) * F])
                nc.sync.dma_start(out=ot[:], in_=ov[:, k * F:(k + 1) * F])
                nc.sync.dma_start(out=wt[:], in_=wv[:, k * F:(k + 1) * F])

                # wo = win * odds (in place into wt); must precede the a
                # computation, which overwrites ot (Tile orders via WAR dep).
                nc.vector.tensor_tensor(wt[:], wt[:], ot[:], op=Alu.mult)
                # a = (odds * 1.1) * probs, in place into ot.
                nc.vector.scalar_tensor_tensor(
                    ot[:], ot[:], float(np.float32(ALPHA)), pt[:],
                    op0=Alu.mult, op1=Alu.mult)

                a3 = ot[:].rearrange("p (n t) -> p n t", t=T)
                w3 = wt[:].rearrange("p (n t) -> p n t", t=T)

                best = ch_pool.tile([P, R], DT, tag="best")
                wob = wob_pool.tile([P, R], DT, tag="wob")
                cmask = ch_pool.tile([P, R], mybir.dt.uint32, tag="cmask")
                junk = ch_pool.tile([P, R], DT, tag="junk")

                # wo_at_best starts as trap 0's value (ScalarE, off DVE path).
                nc.scalar.copy(wob[:], w3[:, :, 0])

                # t=1 fused with init: best = max(a0, a1), c = a1 > a0.
                nc.vector.tensor_tensor(cmask[:], a3[:, :, 1], a3[:, :, 0],
                                        op=Alu.is_gt)
                nc.vector.tensor_tensor(best[:], a3[:, :, 0], a3[:, :, 1],
                                        op=Alu.max)
                nc.vector.copy_predicated(wob[:], cmask[:], w3[:, :, 1])

                for t in range(2, T):
                    nc.vector.tensor_tensor(cmask[:], a3[:, :, t], best[:],
                                            op=Alu.is_gt)
                    nc.vector.copy_predicated(best[:], cmask[:], a3[:, :, t])
                    nc.vector.copy_predicated(wob[:], cmask[:], w3[:, :, t])

                # Stats with free per-partition accumulation.
                nc.vector.scalar_tensor_tensor(
                    junk[:], best[:], 1.0, best[:], op0=Alu.is_gt,
                    op1=Alu.mult, accum_out=acc[:, k:k + 1])
                nc.vector.scalar_tensor_tensor(
                    junk[:], best[:], 1.0, wob[:], op0=Alu.is_gt,
                    op1=Alu.mult, accum_out=acc[:, NT + k:NT + k + 1])
                # With accum_out, op1 is the reduction operator: accum=sum(bet).
                nc.vector.tensor_scalar(
                    junk[:], best[:], 1.0, None, op0=Alu.is_gt, op1=Alu.add,
                    accum_out=acc[:, 2 * NT + k:2 * NT + k + 1])

            nc.sync.dma_start(out=acc_d, in_=acc[:])

    nc.compile()
    return nc


def _get_program():
    global _PROGRAM
    if _PROGRAM is None:
        _PROGRAM = _build_program()
    return _PROGRAM


def _install_ntff_shim():
    """Provide antenv.axon_hooks (missing in this image) so trace=True works.

    Replicates trn_agent_boot's ctypes NTFF hook against libaxon_pjrt.so.
    Only used for profiling runs; plain kernel() calls never need it.
    """
    import contextlib
    import ctypes
    import types

    if "antenv.axon_hooks" in sys.modules:
        return
    try:
        from antenv import axon_hooks  # noqa: F401
        return
    except ImportError:
        pass

    so_path = "/opt/axon/libaxon_pjrt.so"
    hook = None
    try:
        lib = ctypes.CDLL(so_path)
        if hasattr(lib, "axon_start_nrt_profile"):
            lib.axon_start_nrt_profile.argtypes = [
                ctypes.POINTER(ctypes.c_int64), ctypes.c_size_t]
            lib.axon_start_nrt_profile.restype = ctypes.c_int64
            lib.axon_stop_nrt_profile.argtypes = [ctypes.c_char_p]
            lib.axon_stop_nrt_profile.restype = ctypes.c_int64

            @contextlib.contextmanager
            def _hook(output_dir, device_ids):
                import jax
                jax.devices()
                if device_ids:
                    ids = (ctypes.c_int64 * len(device_ids))(*device_ids)
                    rc = lib.axon_start_nrt_profile(ids, len(device_ids))
                else:
                    rc = lib.axon_start_nrt_profile(None, 0)
                if rc != 0:
                    raise RuntimeError(f"axon_start_nrt_profile rc={rc}")
                try:
                    yield
                finally:
                    n = lib.axon_stop_nrt_profile(str(output_dir).encode())
                    print(f"profile: {n} file(s) written to {output_dir}",
                          file=sys.stderr)

            hook = _hook
    except OSError:
        pass

    mod = types.ModuleType("antenv.axon_hooks")
    mod.get_axon_ntff_profile_hook = lambda: hook
    mod.set_axon_ntff_profile_hook = lambda h: None
    sys.modules["antenv.axon_hooks"] = mod


def _run_device(predicted_probs, true_winners, market_odds, trace=False):
    from concourse.bass_utils import run_bass_kernel_spmd

    if trace:
        _install_ntff_shim()
    nc = _get_program()
    in_maps = []
    for i in range(N_CORES):
        s = slice(i * BC, (i + 1) * BC)
        in_maps.append({
            "probs": np.ascontiguousarray(predicted_probs[s]),
            "odds": np.ascontiguousarray(market_odds[s]),
            "win": np.ascontiguousarray(true_winners[s]),
        })
    res = run_bass_kernel_spmd(nc, in_maps, list(range(N_CORES)), trace=trace)
    return res


def kernel(predicted_probs, true_winners, market_odds, _trace=False,
           _result_holder=None):
    res = _run_device(predicted_probs, true_winners, market_odds, trace=_trace)
    if _result_holder is not None:
        _result_holder.append(res)

    S_A = 0.0
    S_WO = 0.0
    NB = 0.0
    for i in range(N_CORES):
        a = res.results[i]["acc"].astype(np.float64)
        S_A += a[:, :NT].sum()
        S_WO += a[:, NT:2 * NT].sum()
        NB += a[:, 2 * NT:].sum()
    num_bets = int(round(NB))

    if num_bets > 0:
        total_expected_profit = PAYOUT_SCALE * (S_A - num_bets)
    else:
        total_expected_profit = -np.float64(
            np.mean(np.max(predicted_probs, axis=1))) * 0.1
    loss = -total_expected_profit / B
    batch_profit = WIN_COEF * S_WO - PAYOUT_SCALE * num_bets

    return (np.float32(loss), np.float32(batch_profit), np.int32(num_bets))


if __name__ == "__main__":
    rng = np.random.default_rng(0)
    n = B
    probs = rng.random((n, T), dtype=np.float32)
    win = (rng.random((n, T)) > 0.8).astype(np.float32)
    odds = rng.random((n, T), dtype=np.float32) * 10.0
    odds[rng.random((n, 1))[:, 0] < 0.1] = 0.0
    out = kernel(probs, win, odds)
    print("kernel out:", out)


# revision 8
# speedup vs baseline: 1.1470x; 1.1470x over previous
"""Trainium2 Bass kernel for the BettingLoss problem.

Strategy (pure data parallel, 8 NeuronCores):
  - Shard the batch dim of the three [B, 6] f32 tensors into 8 contiguous
    row blocks, one per core, and lay each core's shard out trap-major
    [128 partitions, 6 traps, 4096 rows] on the host so every on-chip
    per-trap slice is dense (24B-stride slices pay ~1.75x on the DVE due
    to 16B SBUF cachelines).
  - Per core, stream NT tiles of [128, 6, R] through SBUF; per row:
        a_t  = fl(fl(odds_t * 1.1) * probs_t)      (reference op order)
        wo_t = fl(win_t * odds_t)                   (on GpSimd, off DVE)
        running first-max select chain over t=0..5 tracking
        (best_a, wo_at_best); bet = best_a > 1.0 exactly matches the
        reference's ep > 0 test.
    Per-partition stats via free accum outputs:
        RELU = sum relu(best_a - 1)    (ScalarE)  -> sum_bet_ep / 0.019
        NB   = sum sign(relu)          (ScalarE)  -> num_bets, exact
        S_WO = sum bet * wo_at_best    (DVE stt)  -> batch_profit
  - Host combines partials in float64:
        loss         = -(0.019 * RELU) / B          (fallback if NB == 0)
        batch_profit = (1.1*0.02*0.95) * S_WO - (0.02*0.95) * NB
        num_bets     = NB
"""

import sys

if "/opt/trn_rl_repo" not in sys.path:
    sys.path.insert(0, "/opt/trn_rl_repo")

import numpy as np

B = 4_194_304
T = 6
N_CORES = 8
BC = B // N_CORES          # rows per core
P = 128                    # SBUF partitions
ROWS_PP = BC // P          # rows per partition per core (4096)
R = 1024                   # rows per partition per tile
NT = ROWS_PP // R          # tiles per core
F = T * R                  # free-dim elements per input tile

ALPHA = 1.1
COMMISSION = 0.05
BET_PCT = 0.02
PAYOUT_SCALE = BET_PCT * (1.0 - COMMISSION)          # 0.019
WIN_COEF = ALPHA * BET_PCT * (1.0 - COMMISSION)      # 0.0209

_PROGRAM = None


def _build_program():
    from concourse import bacc, mybir
    from concourse.tile import TileContext

    DT = mybir.dt.float32
    Alu = mybir.AluOpType
    Act = mybir.ActivationFunctionType

    nc = bacc.Bacc("TRN2", target_bir_lowering=False, debug=False,
                   num_devices=N_CORES)
    # Host pre-lays each tensor out as [P, T, ROWS_PP] (trap-major).
    probs_d = nc.dram_tensor("probs", [P, T, ROWS_PP], DT,
                             kind="ExternalInput").ap()
    odds_d = nc.dram_tensor("odds", [P, T, ROWS_PP], DT,
                            kind="ExternalInput").ap()
    win_d = nc.dram_tensor("win", [P, T, ROWS_PP], DT,
                           kind="ExternalInput").ap()
    acc_s_d = nc.dram_tensor("acc_s", [P, 2 * NT], DT,
                             kind="ExternalOutput").ap()
    acc_v_d = nc.dram_tensor("acc_v", [P, NT], DT,
                             kind="ExternalOutput").ap()

    with TileContext(nc) as tc:
        with tc.tile_pool(name="io", bufs=2) as io_pool, \
             tc.tile_pool(name="chain", bufs=1) as ch_pool, \
             tc.tile_pool(name="wobp", bufs=2) as wob_pool, \
             tc.tile_pool(name="accp", bufs=1) as acc_pool:
            acc_s = acc_pool.tile([P, 2 * NT], DT)   # ScalarE: relu, nb
            acc_v = acc_pool.tile([P, NT], DT)       # DVE: s_wo
            neg1 = acc_pool.tile([P, 1], DT)
            nc.vector.memset(neg1[:], -1.0)
            for k in range(NT):
                pt = io_pool.tile([P, F], DT, tag="pt")
                ot = io_pool.tile([P, F], DT, tag="ot")
                wt = io_pool.tile([P, F], DT, tag="wt")
                sl = slice(k * R, (k + 1) * R)
                nc.sync.dma_start(out=pt[:], in_=probs_d[:, :, sl])
                nc.sync.dma_start(out=ot[:], in_=odds_d[:, :, sl])
                nc.sync.dma_start(out=wt[:], in_=win_d[:, :, sl])

                # wo = win * odds in place into wt (GpSimd; only reads ot,
                # so it runs concurrently with the DVE ops below).
                nc.gpsimd.tensor_tensor(wt[:], wt[:], ot[:], op=Alu.mult)
                # a = (odds * 1.1) * probs in place into pt (reads ot).
                nc.vector.scalar_tensor_tensor(
                    pt[:], ot[:], float(np.float32(ALPHA)), pt[:],
                    op0=Alu.mult, op1=Alu.mult)

                a3 = pt[:].rearrange("p (t n) -> p t n", t=T)
                w3 = wt[:].rearrange("p (t n) -> p t n", t=T)

                best = ch_pool.tile([P, R], DT, tag="best")
                wob = wob_pool.tile([P, R], DT, tag="wob")
                cmask = ch_pool.tile([P, R], mybir.dt.uint32, tag="cmask")
                junk = ch_pool.tile([P, R], DT, tag="junk")
                relu_t = wob_pool.tile([P, R], DT, tag="relu")

                # wo_at_best starts as trap 0's value (ScalarE, off DVE).
                nc.scalar.copy(wob[:], w3[:, 0, :])

                # t=1 fused with init: c = a1 > a0, best = max(a0, a1).
                nc.vector.tensor_tensor(cmask[:], a3[:, 1, :], a3[:, 0, :],
                                        op=Alu.is_gt)
                nc.vector.tensor_tensor(best[:], a3[:, 0, :], a3[:, 1, :],
                                        op=Alu.max)
                nc.vector.copy_predicated(wob[:], cmask[:], w3[:, 1, :])

                for t in range(2, T):
                    nc.vector.tensor_tensor(cmask[:], a3[:, t, :], best[:],
                                            op=Alu.is_gt)
                    nc.vector.copy_predicated(best[:], cmask[:], a3[:, t, :])
                    nc.vector.copy_predicated(wob[:], cmask[:], w3[:, t, :])

                # Stats. ScalarE: relu(best-1) summed, then sign(relu) summed
                # (sign(relu) = 1 iff best > 1, so the count is exact).
                nc.scalar.activation(relu_t[:], best[:], Act.Relu,
                                     bias=neg1[:], scale=1.0,
                                     accum_out=acc_s[:, k:k + 1])
                nc.scalar.activation(junk[:], relu_t[:], Act.Sign,
                                     accum_out=acc_s[:, NT + k:NT + k + 1])
                # DVE: sum bet * wo_at_best.
                nc.vector.scalar_tensor_tensor(
                    junk[:], best[:], 1.0, wob[:], op0=Alu.is_gt,
                    op1=Alu.mult, accum_out=acc_v[:, k:k + 1])

            nc.sync.dma_start(out=acc_s_d, in_=acc_s[:])
            nc.sync.dma_start(out=acc_v_d, in_=acc_v[:])

    nc.compile()
    return nc


def _get_program():
    global _PROGRAM
    if _PROGRAM is None:
        _PROGRAM = _build_program()
    return _PROGRAM


def _shard(x, i):
    """Core i's [BC, 6] rows laid out trap-major [P, T, ROWS_PP]."""
    s = x[i * BC:(i + 1) * BC]
    return np.ascontiguousarray(
        s.reshape(P, ROWS_PP, T).transpose(0, 2, 1))


def _install_ntff_shim():
    """Provide antenv.axon_hooks (missing in this image) so trace=True works.

    Replicates trn_agent_boot's ctypes NTFF hook against libaxon_pjrt.so.
    Only used for profiling runs; plain kernel() calls never need it.
    """
    import contextlib
    import ctypes
    import types

    if "antenv.axon_hooks" in sys.modules:
        return
    try:
        from antenv import axon_hooks  # noqa: F401
        return
    except ImportError:
        pass

    so_path = "/opt/axon/libaxon_pjrt.so"
    hook = None
    try:
        lib = ctypes.CDLL(so_path)
        if hasattr(lib, "axon_start_nrt_profile"):
            lib.axon_start_nrt_profile.argtypes = [
                ctypes.POINTER(ctypes.c_int64), ctypes.c_size_t]
            lib.axon_start_nrt_profile.restype = ctypes.c_int64
            lib.axon_stop_nrt_profile.argtypes = [ctypes.c_char_p]
            lib.axon_stop_nrt_profile.restype = ctypes.c_int64

            @contextlib.contextmanager
            def _hook(output_dir, device_ids):
                import jax
                jax.devices()
                if device_ids:
                    ids = (ctypes.c_int64 * len(device_ids))(*device_ids)
                    rc = lib.axon_start_nrt_profile(ids, len(device_ids))
                else:
                    rc = lib.axon_start_nrt_profile(None, 0)
                if rc != 0:
                    raise RuntimeError(f"axon_start_nrt_profile rc={rc}")
                try:
                    yield
                finally:
                    n = lib.axon_stop_nrt_profile(str(output_dir).encode())
                    print(f"profile: {n} file(s) written to {output_dir}",
                          file=sys.stderr)

            hook = _hook
    except OSError:
        pass

    mod = types.ModuleType("antenv.axon_hooks")
    mod.get_axon_ntff_profile_hook = lambda: hook
    mod.set_axon_ntff_profile_hook = lambda h: None
    sys.modules["antenv.axon_hooks"] = mod


def _run_device(predicted_probs, true_winners, market_odds, trace=False):
    from concourse.bass_utils import run_bass_kernel_spmd

    if trace:
        _install_ntff_shim()
    nc = _get_program()
    in_maps = []
    for i in range(N_CORES):
        in_maps.append({
            "probs": _shard(predicted_probs, i),
            "odds": _shard(market_odds, i),
            "win": _shard(true_winners, i),
        })
    res = run_bass_kernel_spmd(nc, in_maps, list(range(N_CORES)), trace=trace)
    return res


def kernel(predicted_probs, true_winners, market_odds, _trace=False,
           _result_holder=None):
    res = _run_device(predicted_probs, true_winners, market_odds, trace=_trace)
    if _result_holder is not None:
        _result_holder.append(res)

    RELU = 0.0
    S_WO = 0.0
    NB = 0.0
    for i in range(N_CORES):
        a_s = res.results[i]["acc_s"].astype(np.float64)
        a_v = res.results[i]["acc_v"].astype(np.float64)
        RELU += a_s[:, :NT].sum()
        NB += a_s[:, NT:].sum()
        S_WO += a_v.sum()
    num_bets = int(round(NB))

    if num_bets > 0:
        total_expected_profit = PAYOUT_SCALE * RELU
    else:
        total_expected_profit = -np.float64(
            np.mean(np.max(predicted_probs, axis=1))) * 0.1
    loss = -total_expected_profit / B
    batch_profit = WIN_COEF * S_WO - PAYOUT_SCALE * num_bets

    return (np.float32(loss), np.float32(batch_profit), np.int32(num_bets))


if __name__ == "__main__":
    rng = np.random.default_rng(0)
    probs = rng.random((B, T), dtype=np.float32)
    win = (rng.random((B, T)) > 0.8).astype(np.float32)
    odds = rng.random((B, T), dtype=np.float32) * 10.0
    odds[rng.random((B, 1))[:, 0] < 0.1] = 0.0
    out = kernel(probs, win, odds)
    print("kernel out:", out)


# revision 10
# speedup vs baseline: 1.2158x; 1.0600x over previous
"""Trainium2 Bass kernel for the BettingLoss problem.

Strategy (pure data parallel, 8 NeuronCores):
  - Shard the batch dim of the three [B, 6] f32 tensors into 8 contiguous
    row blocks, one per core, and lay each core's shard out trap-major
    [128 partitions, 6 traps, 4096 rows] on the host so every on-chip
    per-trap slice is dense (24B-stride slices pay ~1.75x on the DVE due
    to 16B SBUF cachelines).
  - Per core, stream NT tiles of [128, 6, R] through SBUF; per row:
        a_t  = fl(fl(odds_t * 1.1) * probs_t)      (reference op order)
        wo_t = fl(win_t * odds_t)                   (on GpSimd, off DVE)
        running first-max select chain over t=0..5 tracking
        (best_a, wo_at_best); bet = best_a > 1.0 exactly matches the
        reference's ep > 0 test.
    Per-partition stats via free accum outputs:
        RELU = sum relu(best_a - 1)    (ScalarE)  -> sum_bet_ep / 0.019
        NB   = sum sign(relu)          (ScalarE)  -> num_bets, exact
        S_WO = sum bet * wo_at_best    (DVE stt)  -> batch_profit
  - Host combines partials in float64:
        loss         = -(0.019 * RELU) / B          (fallback if NB == 0)
        batch_profit = (1.1*0.02*0.95) * S_WO - (0.02*0.95) * NB
        num_bets     = NB
"""

import sys

if "/opt/trn_rl_repo" not in sys.path:
    sys.path.insert(0, "/opt/trn_rl_repo")

import numpy as np

B = 4_194_304
T = 6
N_CORES = 8
BC = B // N_CORES          # rows per core
P = 128                    # SBUF partitions
ROWS_PP = BC // P          # rows per partition per core (4096)
R = 1024                   # rows per partition per tile
NT = ROWS_PP // R          # tiles per core
F = T * R                  # free-dim elements per input tile

ALPHA = 1.1
COMMISSION = 0.05
BET_PCT = 0.02
PAYOUT_SCALE = BET_PCT * (1.0 - COMMISSION)          # 0.019
WIN_COEF = ALPHA * BET_PCT * (1.0 - COMMISSION)      # 0.0209

_PROGRAM = None


def _build_program():
    from concourse import bacc, mybir
    from concourse.tile import TileContext

    DT = mybir.dt.float32
    Alu = mybir.AluOpType
    Act = mybir.ActivationFunctionType

    nc = bacc.Bacc("TRN2", target_bir_lowering=False, debug=False,
                   num_devices=N_CORES)
    # Host pre-lays each tensor out as [P, T, ROWS_PP] (trap-major).
    probs_d = nc.dram_tensor("probs", [P, T, ROWS_PP], DT,
                             kind="ExternalInput").ap()
    odds_d = nc.dram_tensor("odds", [P, T, ROWS_PP], DT,
                            kind="ExternalInput").ap()
    win_d = nc.dram_tensor("win", [P, T, ROWS_PP], mybir.dt.uint8,
                           kind="ExternalInput").ap()
    acc_s_d = nc.dram_tensor("acc_s", [P, 2 * NT], DT,
                             kind="ExternalOutput").ap()
    acc_v_d = nc.dram_tensor("acc_v", [P, NT], DT,
                             kind="ExternalOutput").ap()

    with TileContext(nc) as tc:
        with tc.tile_pool(name="io", bufs=3) as io_pool, \
             tc.tile_pool(name="chain", bufs=1) as ch_pool, \
             tc.tile_pool(name="wobp", bufs=2) as wob_pool, \
             tc.tile_pool(name="accp", bufs=1) as acc_pool:
            acc_s = acc_pool.tile([P, 2 * NT], DT)   # ScalarE: relu, nb
            acc_v = acc_pool.tile([P, NT], DT)       # DVE: s_wo
            neg1 = acc_pool.tile([P, 1], DT)
            nc.vector.memset(neg1[:], -1.0)
            for k in range(NT):
                pt = io_pool.tile([P, F], DT, tag="pt")
                ot = io_pool.tile([P, F], DT, tag="ot")
                wt = io_pool.tile([P, F], mybir.dt.uint8, tag="wt")
                sl = slice(k * R, (k + 1) * R)
                nc.sync.dma_start(out=pt[:], in_=probs_d[:, :, sl])
                nc.sync.dma_start(out=ot[:], in_=odds_d[:, :, sl])
                nc.sync.dma_start(out=wt[:], in_=win_d[:, :, sl])

                # a = (odds * 1.1) * probs in place into pt (reads ot).
                nc.vector.scalar_tensor_tensor(
                    pt[:], ot[:], float(np.float32(ALPHA)), pt[:],
                    op0=Alu.mult, op1=Alu.mult)
                # wo = win * odds in place into ot (GpSimd; Tile's WAR dep
                # orders this after the a computation above reads ot).
                nc.gpsimd.tensor_tensor(ot[:], wt[:], ot[:], op=Alu.mult)

                a3 = pt[:].rearrange("p (t n) -> p t n", t=T)
                w3 = ot[:].rearrange("p (t n) -> p t n", t=T)

                best = ch_pool.tile([P, R], DT, tag="best")
                wob = wob_pool.tile([P, R], DT, tag="wob")
                cmask = ch_pool.tile([P, R], mybir.dt.uint8, tag="cmask")
                junk = ch_pool.tile([P, R], DT, tag="junk")
                relu_t = wob_pool.tile([P, R], DT, tag="relu")

                # wo_at_best starts as trap 0's value (ScalarE, off DVE).
                nc.scalar.copy(wob[:], w3[:, 0, :])

                # t=1 fused with init: c = a1 > a0, best = max(a0, a1).
                nc.vector.tensor_tensor(cmask[:], a3[:, 1, :], a3[:, 0, :],
                                        op=Alu.is_gt)
                nc.vector.tensor_tensor(best[:], a3[:, 0, :], a3[:, 1, :],
                                        op=Alu.max)
                nc.vector.copy_predicated(wob[:], cmask[:], w3[:, 1, :])

                for t in range(2, T):
                    nc.vector.tensor_tensor(cmask[:], a3[:, t, :], best[:],
                                            op=Alu.is_gt)
                    nc.vector.tensor_tensor(best[:], best[:], a3[:, t, :],
                                            op=Alu.max)
                    nc.vector.copy_predicated(wob[:], cmask[:], w3[:, t, :])

                # Stats. ScalarE: relu(best-1) summed, then sign(relu) summed
                # (sign(relu) = 1 iff best > 1, so the count is exact).
                nc.scalar.activation(relu_t[:], best[:], Act.Relu,
                                     bias=neg1[:], scale=1.0,
                                     accum_out=acc_s[:, k:k + 1])
                nc.scalar.activation(junk[:], relu_t[:], Act.Sign,
                                     accum_out=acc_s[:, NT + k:NT + k + 1])
                # DVE: sum bet * wo_at_best.
                nc.vector.scalar_tensor_tensor(
                    junk[:], best[:], 1.0, wob[:], op0=Alu.is_gt,
                    op1=Alu.mult, accum_out=acc_v[:, k:k + 1])

            nc.sync.dma_start(out=acc_s_d, in_=acc_s[:])
            nc.sync.dma_start(out=acc_v_d, in_=acc_v[:])

    nc.compile()
    return nc


def _get_program():
    global _PROGRAM
    if _PROGRAM is None:
        _PROGRAM = _build_program()
    return _PROGRAM


def _shard(x, i):
    """Core i's [BC, 6] rows laid out trap-major [P, T, ROWS_PP]."""
    s = x[i * BC:(i + 1) * BC]
    out = np.ascontiguousarray(s.reshape(P, ROWS_PP, T).transpose(0, 2, 1))
    return out


def _install_ntff_shim():
    """Provide antenv.axon_hooks (missing in this image) so trace=True works.

    Replicates trn_agent_boot's ctypes NTFF hook against libaxon_pjrt.so.
    Only used for profiling runs; plain kernel() calls never need it.
    """
    import contextlib
    import ctypes
    import types

    if "antenv.axon_hooks" in sys.modules:
        return
    try:
        from antenv import axon_hooks  # noqa: F401
        return
    except ImportError:
        pass

    so_path = "/opt/axon/libaxon_pjrt.so"
    hook = None
    try:
        lib = ctypes.CDLL(so_path)
        if hasattr(lib, "axon_start_nrt_profile"):
            lib.axon_start_nrt_profile.argtypes = [
                ctypes.POINTER(ctypes.c_int64), ctypes.c_size_t]
            lib.axon_start_nrt_profile.restype = ctypes.c_int64
            lib.axon_stop_nrt_profile.argtypes = [ctypes.c_char_p]
            lib.axon_stop_nrt_profile.restype = ctypes.c_int64

            @contextlib.contextmanager
            def _hook(output_dir, device_ids):
                import jax
                jax.devices()
                if device_ids:
                    ids = (ctypes.c_int64 * len(device_ids))(*device_ids)
                    rc = lib.axon_start_nrt_profile(ids, len(device_ids))
                else:
                    rc = lib.axon_start_nrt_profile(None, 0)
                if rc != 0:
                    raise RuntimeError(f"axon_start_nrt_profile rc={rc}")
                try:
                    yield
                finally:
                    n = lib.axon_stop_nrt_profile(str(output_dir).encode())
                    print(f"profile: {n} file(s) written to {output_dir}",
                          file=sys.stderr)

            hook = _hook
    except OSError:
        pass

    mod = types.ModuleType("antenv.axon_hooks")
    mod.get_axon_ntff_profile_hook = lambda: hook
    mod.set_axon_ntff_profile_hook = lambda h: None
    sys.modules["antenv.axon_hooks"] = mod


def _run_device(predicted_probs, true_winners, market_odds, trace=False):
    from concourse.bass_utils import run_bass_kernel_spmd

    if trace:
        _install_ntff_shim()
    nc = _get_program()
    in_maps = []
    for i in range(N_CORES):
        in_maps.append({
            "probs": _shard(predicted_probs, i),
            "odds": _shard(market_odds, i),
            "win": _shard(true_winners, i).astype(np.uint8),
        })
    res = run_bass_kernel_spmd(nc, in_maps, list(range(N_CORES)), trace=trace)
    return res


def kernel(predicted_probs, true_winners, market_odds, _trace=False,
           _result_holder=None):
    res = _run_device(predicted_probs, true_winners, market_odds, trace=_trace)
    if _result_holder is not None:
        _result_holder.append(res)

    RELU = 0.0
    S_WO = 0.0
    NB = 0.0
    for i in range(N_CORES):
        a_s = res.results[i]["acc_s"].astype(np.float64)
        a_v = res.results[i]["acc_v"].astype(np.float64)
        RELU += a_s[:, :NT].sum()
        NB += a_s[:, NT:].sum()
        S_WO += a_v.sum()
    num_bets = int(round(NB))

    if num_bets > 0:
        total_expected_profit = PAYOUT_SCALE * RELU
    else:
        total_expected_profit = -np.float64(
            np.mean(np.max(predicted_probs, axis=1))) * 0.1
    loss = -total_expected_profit / B
    batch_profit = WIN_COEF * S_WO - PAYOUT_SCALE * num_bets

    return (np.float32(loss), np.float32(batch_profit), np.int32(num_bets))


if __name__ == "__main__":
    rng = np.random.default_rng(0)
    probs = rng.random((B, T), dtype=np.float32)
    win = (rng.random((B, T)) > 0.8).astype(np.float32)
    odds = rng.random((B, T), dtype=np.float32) * 10.0
    odds[rng.random((B, 1))[:, 0] < 0.1] = 0.0
    out = kernel(probs, win, odds)
    print("kernel out:", out)


# revision 11
# speedup vs baseline: 1.3492x; 1.1098x over previous
"""Trainium2 Bass kernel for the BettingLoss problem.

Strategy (pure data parallel, 8 NeuronCores):
  - Shard the batch dim of the three [B, 6] f32 tensors into 8 contiguous
    row blocks, one per core, and lay each core's shard out trap-major
    [128 partitions, 6 traps, 4096 rows] on the host so every on-chip
    per-trap slice is dense (24B-stride slices pay ~1.75x on the DVE due
    to 16B SBUF cachelines).
  - Per core, stream NT tiles of [128, 6, R] through SBUF; per row:
        a_t  = fl(fl(odds_t * 1.1) * probs_t)      (reference op order)
        wo_t = fl(win_t * odds_t)                   (on GpSimd, off DVE)
        running first-max select chain over t=0..5 tracking
        (best_a, wo_at_best); bet = best_a > 1.0 exactly matches the
        reference's ep > 0 test.
    Per-partition stats via free accum outputs:
        RELU = sum relu(best_a - 1)    (ScalarE)  -> sum_bet_ep / 0.019
        NB   = sum sign(relu)          (ScalarE)  -> num_bets, exact
        S_WO = sum bet * wo_at_best    (DVE stt)  -> batch_profit
  - Host combines partials in float64:
        loss         = -(0.019 * RELU) / B          (fallback if NB == 0)
        batch_profit = (1.1*0.02*0.95) * S_WO - (0.02*0.95) * NB
        num_bets     = NB
"""

import sys

if "/opt/trn_rl_repo" not in sys.path:
    sys.path.insert(0, "/opt/trn_rl_repo")

import numpy as np

B = 4_194_304
T = 6
N_CORES = 8
BC = B // N_CORES          # rows per core
P = 128                    # SBUF partitions
ROWS_PP = BC // P          # rows per partition per core (4096)
R = 1024                   # rows per partition per tile
NT = ROWS_PP // R          # tiles per core
F = T * R                  # free-dim elements per input tile

ALPHA = 1.1
COMMISSION = 0.05
BET_PCT = 0.02
PAYOUT_SCALE = BET_PCT * (1.0 - COMMISSION)          # 0.019
WIN_COEF = ALPHA * BET_PCT * (1.0 - COMMISSION)      # 0.0209

_PROGRAM = None


def _build_program():
    from concourse import bacc, mybir
    from concourse.tile import TileContext

    DT = mybir.dt.float32
    Alu = mybir.AluOpType
    Act = mybir.ActivationFunctionType

    nc = bacc.Bacc("TRN2", target_bir_lowering=False, debug=False,
                   num_devices=N_CORES)
    # Host pre-lays tensors trap-major; probs+odds packed in one tensor
    # so each tile needs only two dma_starts (one 6.3MB, one 0.8MB).
    po_d = nc.dram_tensor("po", [P, 2, T, ROWS_PP], DT,
                          kind="ExternalInput").ap()
    win_d = nc.dram_tensor("win", [P, T, ROWS_PP], mybir.dt.uint8,
                           kind="ExternalInput").ap()
    acc_s_d = nc.dram_tensor("acc_s", [P, 2 * NT], DT,
                             kind="ExternalOutput").ap()
    acc_v_d = nc.dram_tensor("acc_v", [P, NT], DT,
                             kind="ExternalOutput").ap()

    with TileContext(nc) as tc:
        with tc.tile_pool(name="io", bufs=3) as io_pool, \
             tc.tile_pool(name="chain", bufs=1) as ch_pool, \
             tc.tile_pool(name="wobp", bufs=2) as wob_pool, \
             tc.tile_pool(name="accp", bufs=1) as acc_pool:
            acc_s = acc_pool.tile([P, 2 * NT], DT)   # ScalarE: relu, nb
            acc_v = acc_pool.tile([P, NT], DT)       # DVE: s_wo
            neg1 = acc_pool.tile([P, 1], DT)
            nc.vector.memset(neg1[:], -1.0)
            for k in range(NT):
                pot = io_pool.tile([P, 2 * F], DT, tag="pot")
                wt = io_pool.tile([P, F], mybir.dt.uint8, tag="wt")
                sl = slice(k * R, (k + 1) * R)
                nc.sync.dma_start(out=pot[:], in_=po_d[:, :, :, sl])
                nc.sync.dma_start(out=wt[:], in_=win_d[:, :, sl])

                po4 = pot[:].rearrange("p (c t n) -> p c t n", c=2, t=T)
                pt = po4[:, 0]
                ot = po4[:, 1]
                # a = (odds * 1.1) * probs in place into the probs half.
                nc.vector.scalar_tensor_tensor(
                    pt, ot, float(np.float32(ALPHA)), pt,
                    op0=Alu.mult, op1=Alu.mult)
                # wo = win * odds in place into the odds half (DVE: GpSimd
                # streaming contends with 2-src DVE ops on the shared port).
                nc.vector.tensor_tensor(ot, wt[:], ot, op=Alu.mult)

                a3 = pt
                w3 = ot

                best = ch_pool.tile([P, R], DT, tag="best")
                wob = wob_pool.tile([P, R], DT, tag="wob")
                cmask = ch_pool.tile([P, R], mybir.dt.uint8, tag="cmask")
                junk = ch_pool.tile([P, R], DT, tag="junk")
                relu_t = wob_pool.tile([P, R], DT, tag="relu")

                # wo_at_best starts as trap 0's value.
                nc.vector.tensor_copy(wob[:], w3[:, 0, :])

                # t=1 fused with init: c = a1 > a0, best = max(a0, a1).
                nc.vector.tensor_tensor(cmask[:], a3[:, 1, :], a3[:, 0, :],
                                        op=Alu.is_gt)
                nc.vector.tensor_tensor(best[:], a3[:, 0, :], a3[:, 1, :],
                                        op=Alu.max)
                nc.vector.copy_predicated(wob[:], cmask[:], w3[:, 1, :])

                for t in range(2, T):
                    nc.vector.tensor_tensor(cmask[:], a3[:, t, :], best[:],
                                            op=Alu.is_gt)
                    nc.vector.tensor_tensor(best[:], best[:], a3[:, t, :],
                                            op=Alu.max)
                    nc.vector.copy_predicated(wob[:], cmask[:], w3[:, t, :])

                # Stats. ScalarE: relu(best-1) summed, then sign(relu) summed
                # (sign(relu) = 1 iff best > 1, so the count is exact).
                nc.scalar.activation(relu_t[:], best[:], Act.Relu,
                                     bias=neg1[:], scale=1.0,
                                     accum_out=acc_s[:, k:k + 1])
                nc.scalar.activation(junk[:], relu_t[:], Act.Sign,
                                     accum_out=acc_s[:, NT + k:NT + k + 1])
                # DVE: sum bet * wo_at_best.
                nc.vector.scalar_tensor_tensor(
                    junk[:], best[:], 1.0, wob[:], op0=Alu.is_gt,
                    op1=Alu.mult, accum_out=acc_v[:, k:k + 1])

            nc.sync.dma_start(out=acc_s_d, in_=acc_s[:])
            nc.sync.dma_start(out=acc_v_d, in_=acc_v[:])

    nc.compile()
    return nc


def _get_program():
    global _PROGRAM
    if _PROGRAM is None:
        _PROGRAM = _build_program()
    return _PROGRAM


def _shard(x, i):
    """Core i's [BC, 6] rows laid out trap-major [P, T, ROWS_PP]."""
    s = x[i * BC:(i + 1) * BC]
    out = np.ascontiguousarray(s.reshape(P, ROWS_PP, T).transpose(0, 2, 1))
    return out


def _shard_po(probs, odds, i):
    """probs+odds packed [P, 2, T, ROWS_PP] for one 6.3MB dma per tile."""
    out = np.empty((P, 2, T, ROWS_PP), np.float32)
    for j, x in enumerate((probs, odds)):
        s = x[i * BC:(i + 1) * BC]
        out[:, j] = s.reshape(P, ROWS_PP, T).transpose(0, 2, 1)
    return out


def _install_ntff_shim():
    """Provide antenv.axon_hooks (missing in this image) so trace=True works.

    Replicates trn_agent_boot's ctypes NTFF hook against libaxon_pjrt.so.
    Only used for profiling runs; plain kernel() calls never need it.
    """
    import contextlib
    import ctypes
    import types

    if "antenv.axon_hooks" in sys.modules:
        return
    try:
        from antenv import axon_hooks  # noqa: F401
        return
    except ImportError:
        pass

    so_path = "/opt/axon/libaxon_pjrt.so"
    hook = None
    try:
        lib = ctypes.CDLL(so_path)
        if hasattr(lib, "axon_start_nrt_profile"):
            lib.axon_start_nrt_profile.argtypes = [
                ctypes.POINTER(ctypes.c_int64), ctypes.c_size_t]
            lib.axon_start_nrt_profile.restype = ctypes.c_int64
            lib.axon_stop_nrt_profile.argtypes = [ctypes.c_char_p]
            lib.axon_stop_nrt_profile.restype = ctypes.c_int64

            @contextlib.contextmanager
            def _hook(output_dir, device_ids):
                import jax
                jax.devices()
                if device_ids:
                    ids = (ctypes.c_int64 * len(device_ids))(*device_ids)
                    rc = lib.axon_start_nrt_profile(ids, len(device_ids))
                else:
                    rc = lib.axon_start_nrt_profile(None, 0)
                if rc != 0:
                    raise RuntimeError(f"axon_start_nrt_profile rc={rc}")
                try:
                    yield
                finally:
                    n = lib.axon_stop_nrt_profile(str(output_dir).encode())
                    print(f"profile: {n} file(s) written to {output_dir}",
                          file=sys.stderr)

            hook = _hook
    except OSError:
        pass

    mod = types.ModuleType("antenv.axon_hooks")
    mod.get_axon_ntff_profile_hook = lambda: hook
    mod.set_axon_ntff_profile_hook = lambda h: None
    sys.modules["antenv.axon_hooks"] = mod


def _run_device(predicted_probs, true_winners, market_odds, trace=False):
    from concourse.bass_utils import run_bass_kernel_spmd

    if trace:
        _install_ntff_shim()
    nc = _get_program()
    in_maps = []
    for i in range(N_CORES):
        in_maps.append({
            "po": _shard_po(predicted_probs, market_odds, i),
            "win": _shard(true_winners, i).astype(np.uint8),
        })
    res = run_bass_kernel_spmd(nc, in_maps, list(range(N_CORES)), trace=trace)
    return res


def kernel(predicted_probs, true_winners, market_odds, _trace=False,
           _result_holder=None):
    res = _run_device(predicted_probs, true_winners, market_odds, trace=_trace)
    if _result_holder is not None:
        _result_holder.append(res)

    RELU = 0.0
    S_WO = 0.0
    NB = 0.0
    for i in range(N_CORES):
        a_s = res.results[i]["acc_s"].astype(np.float64)
        a_v = res.results[i]["acc_v"].astype(np.float64)
        RELU += a_s[:, :NT].sum()
        NB += a_s[:, NT:].sum()
        S_WO += a_v.sum()
    num_bets = int(round(NB))

    if num_bets > 0:
        total_expected_profit = PAYOUT_SCALE * RELU
    else:
        total_expected_profit = -np.float64(
            np.mean(np.max(predicted_probs, axis=1))) * 0.1
    loss = -total_expected_profit / B
    batch_profit = WIN_COEF * S_WO - PAYOUT_SCALE * num_bets

    return (np.float32(loss), np.float32(batch_profit), np.int32(num_bets))


if __name__ == "__main__":
    rng = np.random.default_rng(0)
    probs = rng.random((B, T), dtype=np.float32)
    win = (rng.random((B, T)) > 0.8).astype(np.float32)
    odds = rng.random((B, T), dtype=np.float32) * 10.0
    odds[rng.random((B, 1))[:, 0] < 0.1] = 0.0
    out = kernel(probs, win, odds)
    print("kernel out:", out)
